# revision 1
# baseline (speedup 1.0000x reference)
"""Distributed 3-layer GAT + FC kernel for Trainium2 (8 NeuronCores).

Strategy (graph/data parallel, per the sharding hint):
  - Nodes are assigned to the 8 cores by in-degree rank interleaving
    (rank r -> core r%8, local slot r//8), so every core gets an almost
    identical degree profile and per-tile gather counts line up across
    cores (the SPMD program is shared).
  - Each core owns its nodes' incoming edges.  Edges are laid out
    degree-bucketed: dst node = partition, incoming-edge slot = free-dim
    column.  One indirect DMA (128 rows, one per partition) gathers the
    source-node table rows for one edge-slot column.
  - Per layer, each core projects its own nodes ([h | el | er] in one
    fused matmul, the el/er attention reductions are folded into the
    projection matrix host-side), then an AllGather replicates the
    [h | el] node table to every core (the "halo exchange").
  - Edge softmax: the segment max is skipped (|e| <= ~7 on this data,
    exp cannot overflow) and the alpha normalization is folded into a
    single divide after the weighted segment sum:
        out[n] = (sum_e exp(e_e) h[src_e]) / (sum_e exp(e_e)).
    Padding edge slots point at a dummy table row with el = -1e30 so
    exp() contributes exactly 0.
  - Small weight tensors are replicated; the final FC stays node-local.
"""

import numpy as np

N_NODES = 50000
N_EDGES = 1000000
NC = 8
NPC = N_NODES // NC          # 6250 owned nodes per core
NT = 49                      # node tiles per core (128 nodes each)
NPCP = NT * 128              # 6272 padded nodes per core
NTOT = NPCP * NC             # 50176 table rows (without dummy)
DUMMY = NTOT                 # dummy row index
TBL = NTOT + 1
NEG_SLOPE = 0.2

# (Fin, H, D) per GAT layer
LAYERS = [(25, 4, 10), (40, 4, 25), (100, 1, 50)]

_cache = {}


def _patch_tile_drain():
    """walrus in this toolchain rejects instructions carrying more than one
    semaphore wait; split the TileContext tail drain's waits onto
    single-wait NOPs."""
    import concourse.tile as tile_mod
    import concourse.mybir as mybir
    from concourse.vector_clock import ScopedClock

    if getattr(tile_mod.TileContext, "_drain_patched", False):
        return

    def _patched(self, tick_clock, wait_clock):
        nc = self.nc
        drain_inst = nc.sync.drain()
        wait_clock.add_sem_waits(
            drain_inst.ins, ScopedClock({None: tick_clock.global_clock})
        )
        si = drain_inst.ins.sync_info
        waits = list(si.on_wait or []) if si is not None else []
        if len(waits) > 1:
            si.on_wait.clear()
            bb = nc.cur_bb.bb
            assert bb.instructions[-1] is drain_inst.ins
            bb.instructions.pop()
            for w in waits:
                nop = nc.sync.nop(nofuse=True, hint="drain_wait_split")
                if nop.ins.sync_info is None:
                    nop.ins.sync_info = mybir.SyncInfo(on_wait=[w], on_update=[])
                else:
                    nop.ins.sync_info.on_wait.append(w)
            bb.add_instruction(drain_inst.ins)
        nc.all_engine_barrier()
        assert self.sems is not None
        popped = nc._tile_sem_poison_stack.pop()
        assert popped is self._sem_poison
        nc.clear_and_free_semaphores(list(self.sems.allocated().values()))
        nc.all_engine_barrier()

    tile_mod.TileContext._drain_and_barrier = _patched
    tile_mod.TileContext._drain_patched = True


def _preprocess(src, dst):
    """Node->core assignment, degree layout, per-column gather indices."""
    deg = np.bincount(dst, minlength=N_NODES)
    rank = np.argsort(-deg, kind="stable")
    node_core = np.empty(N_NODES, np.int64)
    node_loc = np.empty(N_NODES, np.int64)
    node_core[rank] = np.arange(N_NODES) % NC
    node_loc[rank] = np.arange(N_NODES) // NC
    glob = node_core * NPCP + node_loc

    loc_deg = np.zeros((NC, NPCP), np.int64)
    loc_deg[node_core, node_loc] = deg
    K_t = [int(loc_deg[:, t * 128:(t + 1) * 128].max()) for t in range(NT)]
    colbase = np.concatenate([[0], np.cumsum(K_t)[:-1]]).astype(np.int64)
    C_tot = int(sum(K_t))

    idx_arr = np.full((NC, 128, C_tot), DUMMY, np.int32)
    key = node_core[dst] * NPCP + node_loc[dst]
    eorder = np.argsort(key, kind="stable")
    ssorted = src[eorder]
    ksorted = key[eorder]
    starts = np.searchsorted(ksorted, np.arange(NC * NPCP))
    ends = np.searchsorted(ksorted, np.arange(NC * NPCP) + 1)
    gsorted = glob[ssorted]
    for c in range(NC):
        base = c * NPCP
        for t in range(NT):
            cb = colbase[t]
            for u in range(128):
                loc = base + t * 128 + u
                s0, s1 = starts[loc], ends[loc]
                if s1 > s0:
                    idx_arr[c, u, cb:cb + (s1 - s0)] = gsorted[s0:s1]
    return node_core, node_loc, K_t, colbase, C_tot, idx_arr


def _proj_matrix(W, al, ar):
    """P = [W; L^T W; R^T W] so that P @ x = [h; el; er] (feature-major)."""
    H, D = al.shape
    HD = H * D
    L = np.zeros((HD, H), np.float32)
    R = np.zeros((HD, H), np.float32)
    for h in range(H):
        L[h * D:(h + 1) * D, h] = al[h]
        R[h * D:(h + 1) * D, h] = ar[h]
    return np.vstack([W, L.T @ W, R.T @ W]).astype(np.float32)


def _build(K_t, C_tot):
    import concourse.bass as bass
    import concourse.bacc as bacc
    import concourse.mybir as mybir
    from concourse.tile import TileContext
    from concourse.masks import make_identity

    _patch_tile_drain()

    nc = bacc.Bacc("TRN2", target_bir_lowering=False, debug=False, num_devices=NC)
    f32 = mybir.dt.float32

    xin = nc.dram_tensor("xin", [25, NPCP], f32, kind="ExternalInput")
    idx = nc.dram_tensor("idx", [128, C_tot], mybir.dt.int32, kind="ExternalInput")
    pTs = [nc.dram_tensor(f"pT{l}", [LAYERS[l][0], LAYERS[l][1] * LAYERS[l][2] + 2 * LAYERS[l][1]],
                          f32, kind="ExternalInput") for l in range(3)]
    bts = [nc.dram_tensor(f"bias{l}", [128, LAYERS[l][1] * LAYERS[l][2]],
                          f32, kind="ExternalInput") for l in range(3)]
    fcT = nc.dram_tensor("fcT", [50, 93], f32, kind="ExternalInput")
    fcb = nc.dram_tensor("fcb", [93, 1], f32, kind="ExternalInput")
    out93 = nc.dram_tensor("out93", [93, NPCP], f32, kind="ExternalOutput")

    # internal DRAM
    xd = [None,
          nc.dram_tensor("x2", [40, NPCP], f32, kind="Internal"),
          nc.dram_tensor("x3", [100, NPCP], f32, kind="Internal"),
          nc.dram_tensor("x4", [50, NPCP], f32, kind="Internal")]
    pieces = []
    tables = []
    for l, (Fin, H, D) in enumerate(LAYERS):
        DR = H * D + H
        pieces.append(nc.dram_tensor(f"piece{l}", [NPCP, DR], f32, kind="Internal"))
        tables.append(nc.dram_tensor(f"table{l}", [TBL, DR], f32, kind="Internal",
                                     addr_space="Shared"))

    with TileContext(nc) as tc:
        with (
            tc.tile_pool(name="const", bufs=1) as cpool,
            tc.tile_pool(name="io", bufs=3) as iop,
            tc.tile_pool(name="gt", bufs=4) as gtp,
            tc.tile_pool(name="wk", bufs=3) as wkp,
            tc.tile_pool(name="ps", bufs=2, space="PSUM") as psp,
        ):
            ident = cpool.tile([128, 128], f32, tag="ident")
            make_identity(nc, ident[:])
            it = cpool.tile([128, C_tot], mybir.dt.int32, tag="idx")
            nc.sync.dma_start(it[:], idx[:])
            pt_t = []
            b_t = []
            for l, (Fin, H, D) in enumerate(LAYERS):
                HD = H * D
                p = cpool.tile([Fin, HD + 2 * H], f32, tag=f"pt{l}")
                nc.sync.dma_start(p[:], pTs[l][:])
                pt_t.append(p)
                b = cpool.tile([128, HD], f32, tag=f"b{l}")
                nc.sync.dma_start(b[:], bts[l][:])
                b_t.append(b)
            fct = cpool.tile([50, 93], f32, tag="fct")
            nc.sync.dma_start(fct[:], fcT[:])
            fcbt = cpool.tile([93, 1], f32, tag="fcbt")
            nc.sync.dma_start(fcbt[:], fcb[:])
            ers = [cpool.tile([128, NT, LAYERS[l][1]], f32, tag=f"er{l}",
                              name=f"er{l}") for l in range(3)]

            for l, (Fin, H, D) in enumerate(LAYERS):
                HD = H * D
                DR = HD + H
                PR = HD + 2 * H
                xsrc = xin if l == 0 else xd[l]
                table = tables[l]
                piece = pieces[l]
                er_sb = ers[l]

                # ---- projection of own nodes + table piece write ----
                for s in range(NT):
                    xs_t = iop.tile([Fin, 128], f32, tag="xs")
                    nc.sync.dma_start(xs_t[:], xsrc[:, s * 128:(s + 1) * 128])
                    cp = psp.tile([PR, 128], f32, tag="ps_a", space="PSUM")
                    nc.tensor.matmul(cp[:], lhsT=pt_t[l][:], rhs=xs_t[:],
                                     start=True, stop=True)
                    cs = wkp.tile([PR, 128], f32, tag="cs")
                    nc.vector.tensor_copy(cs[:], cp[:])
                    gp = psp.tile([128, PR], f32, tag="ps_b", space="PSUM")
                    nc.tensor.transpose(gp[:], cs[:], ident[:PR, :PR])
                    gs = wkp.tile([128, PR], f32, tag="gs")
                    nc.vector.tensor_copy(gs[:], gp[:])
                    nc.sync.dma_start(piece[s * 128:(s + 1) * 128, :], gs[:, 0:DR])
                    nc.vector.tensor_copy(er_sb[:, s, :], gs[:, DR:PR])

                # ---- halo exchange ----
                nc.gpsimd.collective_compute(
                    "AllGather", mybir.AluOpType.bypass,
                    replica_groups=[list(range(NC))],
                    ins=[piece[:]], outs=[table[0:NTOT, :]],
                )
                dum = wkp.tile([1, DR], f32, tag="dum")
                nc.vector.memset(dum[:, 0:HD], 0.0)
                nc.vector.memset(dum[:, HD:DR], -1e30)
                nc.sync.dma_start(table[NTOT:TBL, :], dum[:])

                # ---- edge phase ----
                col = 0
                for t in range(NT):
                    K = K_t[t]
                    gt = gtp.tile([128, K, DR], f32, tag="gt")
                    for k in range(K):
                        nc.gpsimd.indirect_dma_start(
                            out=gt[:, k, :], out_offset=None, in_=table[:],
                            in_offset=bass.IndirectOffsetOnAxis(
                                ap=it[:, col + k:col + k + 1], axis=0),
                        )
                    den = wkp.tile([128, H], f32, tag="den")
                    acc = wkp.tile([128, HD], f32, tag="acc")
                    e_h = [wkp.tile([128, K], f32, tag=f"e{h}", name=f"eh{h}")
                           for h in range(H)]
                    for h in range(H):
                        nc.scalar.activation(
                            e_h[h][:], gt[:, :, HD + h],
                            mybir.ActivationFunctionType.Prelu,
                            bias=er_sb[:, t, h:h + 1], alpha=NEG_SLOPE)
                    for h in range(H):
                        nc.scalar.activation(
                            e_h[h][:], e_h[h][:],
                            mybir.ActivationFunctionType.Exp,
                            accum_out=den[:, h:h + 1])
                    for h in range(H):
                        m_h = wkp.tile([128, K, D], f32, tag="m")
                        nc.vector.tensor_tensor(
                            out=m_h[:], in0=gt[:, :, h * D:(h + 1) * D],
                            in1=e_h[h][:, :, None].broadcast_to([128, K, D]),
                            op=mybir.AluOpType.mult)
                        nc.vector.tensor_reduce(
                            out=acc[:, h * D:(h + 1) * D],
                            in_=m_h[:].rearrange("p k d -> p d k"),
                            axis=mybir.AxisListType.X, op=mybir.AluOpType.add)
                    nc.vector.tensor_scalar_max(den[:], den[:], 1e-30)
                    rden = wkp.tile([128, H], f32, tag="rden")
                    nc.vector.reciprocal(rden[:], den[:])
                    o = wkp.tile([128, HD], f32, tag="o")
                    nc.vector.tensor_tensor(
                        out=o[:].rearrange("p (h d) -> p h d", h=H),
                        in0=acc[:].rearrange("p (h d) -> p h d", h=H),
                        in1=rden[:, :, None].broadcast_to([128, H, D]),
                        op=mybir.AluOpType.mult)
                    nc.vector.tensor_add(o[:], o[:], b_t[l][:])
                    nc.vector.tensor_scalar_max(o[:], o[:], 0.0)
                    xp = psp.tile([HD, 128], f32, tag="ps_b", space="PSUM")
                    nc.tensor.transpose(xp[:], o[:], ident[:])
                    xs2 = wkp.tile([HD, 128], f32, tag="xs2")
                    nc.vector.tensor_copy(xs2[:], xp[:])
                    nc.sync.dma_start(xd[l + 1][:, t * 128:(t + 1) * 128], xs2[:])
                    col += K

            # ---- final FC ----
            for s in range(NT):
                xs_t = iop.tile([50, 128], f32, tag="xs")
                nc.sync.dma_start(xs_t[:], xd[3][:, s * 128:(s + 1) * 128])
                fp = psp.tile([93, 128], f32, tag="ps_a", space="PSUM")
                nc.tensor.matmul(fp[:], lhsT=fct[:], rhs=xs_t[:],
                                 start=True, stop=True)
                fo = wkp.tile([93, 128], f32, tag="fo")
                nc.vector.tensor_tensor(
                    out=fo[:], in0=fp[:],
                    in1=fcbt[:, 0:1].broadcast_to([93, 128]),
                    op=mybir.AluOpType.add)
                nc.sync.dma_start(out93[:, s * 128:(s + 1) * 128], fo[:])

    nc.compile()
    return nc


def kernel(**inputs):
    from concourse import bass_utils

    src = np.ascontiguousarray(np.asarray(inputs["src"], dtype=np.int32))
    dst = np.ascontiguousarray(np.asarray(inputs["dst"], dtype=np.int32))
    feats = np.asarray(inputs["features"], dtype=np.float32)

    node_core, node_loc, K_t, colbase, C_tot, idx_arr = _preprocess(src, dst)

    ck = (tuple(K_t), C_tot)
    if ck not in _cache:
        _cache[ck] = _build(K_t, C_tot)
    nc = _cache[ck]

    # host-side tensors
    pTl = []
    btl = []
    for l in range(3):
        W = np.asarray(inputs[f"W{l + 1}"], np.float32)
        al = np.asarray(inputs[f"al{l + 1}"], np.float32)
        ar = np.asarray(inputs[f"ar{l + 1}"], np.float32)
        b = np.asarray(inputs[f"b{l + 1}"], np.float32)
        P = _proj_matrix(W, al, ar)          # [PR, Fin]
        pTl.append(np.ascontiguousarray(P.T))  # [Fin, PR]
        btl.append(np.ascontiguousarray(np.tile(b[None, :], (128, 1))))
    fcw = np.asarray(inputs["fc_w"], np.float32)      # [93, 50]
    fcb = np.asarray(inputs["fc_b"], np.float32).reshape(93, 1)
    fcT = np.ascontiguousarray(fcw.T)                 # [50, 93]

    in_maps = []
    for c in range(NC):
        xfm = np.zeros((25, NPCP), np.float32)
        sel = node_core == c
        xfm[:, node_loc[sel]] = feats[sel].T
        m = {"xin": xfm, "idx": np.ascontiguousarray(idx_arr[c]),
             "fcT": fcT, "fcb": fcb}
        for l in range(3):
            m[f"pT{l}"] = pTl[l]
            m[f"bias{l}"] = btl[l]
        in_maps.append(m)

    res = bass_utils.run_bass_kernel_spmd(nc, in_maps, core_ids=list(range(NC)))

    out = np.zeros((N_NODES, 93), np.float32)
    for c in range(NC):
        o = res.results[c]["out93"]          # [93, NPCP]
        sel = node_core == c
        out[np.where(sel)[0]] = o[:, node_loc[sel]].T
    return out



# revision 21
# speedup vs baseline: 1.1236x; 1.1236x over previous
"""Distributed 3-layer GAT + FC kernel for Trainium2 (8 NeuronCores).

Strategy (graph/data parallel per the sharding hint):
  - Nodes are assigned to cores band-by-band (8 nodes of equal in-degree
    rank per band); within a band, members with high OUT-degree go to the
    cores whose table rows fall in multiple gather windows (see below).
  - Per layer, each core projects its own nodes ([h | el | er] in one
    fused matmul), then an AllGather replicates the [h | el] table rows
    (bf16, padded to 128 elems = 256 B) to every core.
  - Edge gathers use dma_gather (InstDMAGatherAnt): one instruction
    gathers thousands of rows (the per-instruction SWDGE fixed cost made
    per-edge-slot indirect DMAs the old bottleneck).  dma_gather indices
    are int16, so a 50176-row table is covered by THREE overlapping
    32768-row windows; each destination's edge slots are split across the
    three windows, and slots in overlap regions are balanced to minimize
    the per-tile max column counts.
  - Padding slots gather a harmless row and are zeroed by a static mask
    folded into the edge softmax numerator (alpha normalization is a
    single divide after the weighted segment sum; segment max is skipped,
    |e| is small on this data).
  - The projection for layer l+1 and the final FC are fused into layer
    l's edge loop (TensorE is otherwise idle), so activations never
    round-trip through DRAM.
"""

import numpy as np
import ml_dtypes

N_NODES = 50000
N_EDGES = 1000000
NC = 8
NT = 49                      # node tiles per core (128 nodes each)
NPCP = NT * 128              # 6272 padded nodes per core
NB = N_NODES // NC           # 6250 bands
NTOT = NPCP * NC             # 50176 table rows
WIN = 32768
WSTARTS = (0, (NTOT - WIN) // 2, NTOT - WIN)      # 0, 8704, 17408
ELEM = 128                   # bf16 elems per table row (256 B)
NEG_SLOPE = 0.2
COLS_BUDGET = 110            # max gather-buffer columns per tile group
CHUNK_COLS = 96              # max columns per dma_gather (96*128 idxs = 769
                             # descs/engine verified OK with multi-packet)
SINGLE_PACKET = False        # single_packet caps a gather at 64 descs/engine

# (Fin, H, D) per GAT layer
LAYERS = [(25, 4, 10), (40, 4, 25), (100, 1, 50)]

_cache = {}


def _patch_tile_drain():
    """walrus in this toolchain rejects instructions carrying more than one
    semaphore wait; split the TileContext tail drain's waits onto
    single-wait NOPs."""
    import concourse.tile as tile_mod
    import concourse.mybir as mybir
    from concourse.vector_clock import ScopedClock

    if getattr(tile_mod.TileContext, "_drain_patched", False):
        return

    def _patched(self, tick_clock, wait_clock):
        nc = self.nc
        drain_inst = nc.sync.drain()
        wait_clock.add_sem_waits(
            drain_inst.ins, ScopedClock({None: tick_clock.global_clock})
        )
        si = drain_inst.ins.sync_info
        waits = list(si.on_wait or []) if si is not None else []
        if len(waits) > 1:
            si.on_wait.clear()
            bb = nc.cur_bb.bb
            assert bb.instructions[-1] is drain_inst.ins
            bb.instructions.pop()
            for w in waits:
                nop = nc.sync.nop(nofuse=True, hint="drain_wait_split")
                if nop.ins.sync_info is None:
                    nop.ins.sync_info = mybir.SyncInfo(on_wait=[w], on_update=[])
                else:
                    nop.ins.sync_info.on_wait.append(w)
            bb.add_instruction(drain_inst.ins)
        nc.all_engine_barrier()
        assert self.sems is not None
        popped = nc._tile_sem_poison_stack.pop()
        assert popped is self._sem_poison
        nc.clear_and_free_semaphores(list(self.sems.allocated().values()))
        nc.all_engine_barrier()

    tile_mod.TileContext._drain_and_barrier = _patched
    tile_mod.TileContext._drain_patched = True


def _cov(rows):
    c = np.zeros(rows.shape, np.int64)
    for ws in WSTARTS:
        c += (rows >= ws) & (rows < ws + WIN)
    return c


def _best_split(a0, e01, e012, e12, a2, deg, slack=14):
    """Split each node's edge slots across windows 0/1/2 with shared
    per-window column counts.  Returns (K0, K1, K2, L0, L2)."""
    K = int(deg.max()) if deg.size else 0
    L0lo, L2lo = int(a0.max()), int(a2.max())
    best = None
    for L0 in range(L0lo, min(L0lo + slack, K) + 1):
        u01 = np.minimum(e01, np.maximum(0, L0 - a0))
        u012 = np.minimum(e012, np.maximum(0, L0 - a0 - u01))
        g0 = a0 + u01 + u012
        K0 = int(g0.max())
        r012 = e012 - u012
        for L2 in range(L2lo, min(L2lo + slack, K) + 1):
            v12 = np.minimum(e12, np.maximum(0, L2 - a2))
            v012 = np.minimum(r012, np.maximum(0, L2 - a2 - v12))
            g2 = a2 + v12 + v012
            K2 = int(g2.max())
            K1 = int((deg - g0 - g2).max())
            tot = K0 + K1 + K2
            if best is None or tot < best[0]:
                best = (tot, K0, K1, K2, L0, L2)
    return best


def _preprocess(src, dst):
    deg = np.bincount(dst, minlength=N_NODES)
    odeg = np.bincount(src, minlength=N_NODES)
    rank = np.argsort(-deg, kind="stable")

    # band assignment: high out-degree members -> high window-coverage cores
    members = rank.reshape(NB, NC)
    rows = np.arange(NC)[None, :] * NPCP + np.arange(NB)[:, None]
    corder = np.argsort(-_cov(rows), axis=1, kind="stable")
    morder = np.take_along_axis(
        members, np.argsort(-odeg[members], axis=1, kind="stable"), axis=1)
    core = np.empty(N_NODES, np.int64)
    loc = np.empty(N_NODES, np.int64)
    core[morder.ravel()] = corder.ravel()
    loc[morder.ravel()] = np.repeat(np.arange(NB), NC)
    glob = core * NPCP + loc

    srow = glob[src]
    nid = glob[dst]
    b1, b2, b3, b4 = WSTARTS[1], WSTARTS[2], WSTARTS[0] + WIN, WSTARTS[1] + WIN
    cls = np.full(N_EDGES, 2, np.int8)          # flex012
    cls[srow < b1] = 0                          # strict w0
    cls[(srow >= b1) & (srow < b2)] = 1         # flex01
    cls[(srow >= b3) & (srow < b4)] = 3         # flex12
    cls[srow >= b4] = 4                         # strict w2

    cnt = np.zeros((5, NTOT), np.int64)
    for m in range(5):
        np.add.at(cnt[m], nid[cls == m], 1)
    a0, e01, e012, e12, a2 = cnt
    degg = cnt.sum(axis=0)                      # per glob id degree

    tile_of = np.arange(NTOT) % NPCP // 128

    # per-tile column estimate -> variable grouping
    K3_t = []
    for t in range(NT):
        m = tile_of == t
        K3_t.append(_best_split(a0[m], e01[m], e012[m], e12[m], a2[m], degg[m])[0])
    groups = []
    t0 = 0
    while t0 < NT:
        g = 1
        while t0 + g < NT and (g + 1) * max(K3_t[t0:t0 + g + 1]) <= COLS_BUDGET:
            g += 1
        groups.append((t0, g))
        t0 += g

    # per-group split + per-node assignments
    meta = []          # (t0, G, (K0,K1,K2), ioffs, boffs, moff)
    u01a = np.zeros(NTOT, np.int64)
    u012a = np.zeros(NTOT, np.int64)
    v12a = np.zeros(NTOT, np.int64)
    v012a = np.zeros(NTOT, np.int64)
    ioff = 0
    moff = 0
    for (t0, G) in groups:
        m = (tile_of >= t0) & (tile_of < t0 + G)
        tot, K0, K1, K2, L0, L2 = _best_split(
            a0[m], e01[m], e012[m], e12[m], a2[m], degg[m])
        u01 = np.minimum(e01[m], np.maximum(0, L0 - a0[m]))
        u012 = np.minimum(e012[m], np.maximum(0, L0 - a0[m] - u01))
        v12 = np.minimum(e12[m], np.maximum(0, L2 - a2[m]))
        v012 = np.minimum(e012[m] - u012, np.maximum(0, L2 - a2[m] - v12))
        u01a[m], u012a[m], v12a[m], v012a[m] = u01, u012, v12, v012
        Ks = (K0, K1, K2)
        ioffs = []
        boffs = []
        chunks = []        # (w, boff_cols, ncols, ioff) per dma_gather
        b = 0
        for w in range(3):
            ioffs.append(ioff)
            boffs.append(b)
            cw = G * Ks[w]
            c0 = 0
            while c0 < cw:
                nc_ = min(CHUNK_COLS, cw - c0)
                chunks.append((w, b + c0, nc_, ioff + c0 * 8))
                c0 += nc_
            ioff += cw * 8
            b += cw
        meta.append((t0, G, Ks, tuple(ioffs), tuple(boffs), moff,
                     tuple(chunks)))
        moff += b
    C3p = moff
    IW = ioff

    # edge lists per glob id, ordered by class
    eorder = np.lexsort((cls, nid))
    s_sorted = srow[eorder]
    c_sorted = cls[eorder]
    starts = np.searchsorted(nid[eorder], np.arange(NTOT))
    ends = np.searchsorted(nid[eorder], np.arange(NTOT) + 1)

    flat = np.zeros((NC, IW * 16), np.int16)        # flat j-order indices
    mask = np.zeros((NC, 128, C3p), np.float32)

    for (t0, G, Ks, ioffs, boffs, moff_g, chunks) in meta:
        K0, K1, K2 = Ks
        for c in range(NC):
            base_g = c * NPCP + t0 * 128
            for trel in range(G):
                for p in range(128):
                    gid = base_g + trel * 128 + p
                    st, en = starts[gid], ends[gid]
                    if en <= st:
                        continue
                    es = s_sorted[st:en]
                    ec = c_sorted[st:en]
                    bnd = np.searchsorted(ec, [1, 2, 3, 4])
                    l_s0, l_01, l_012, l_12, l_s2 = (
                        es[:bnd[0]], es[bnd[0]:bnd[1]], es[bnd[1]:bnd[2]],
                        es[bnd[2]:bnd[3]], es[bnd[3]:])
                    u01, u012 = u01a[gid], u012a[gid]
                    v12, v012 = v12a[gid], v012a[gid]
                    n012 = len(l_012)
                    g0 = np.concatenate([l_s0, l_01[:u01], l_012[:u012]])
                    g2 = np.concatenate([l_s2, l_12[:v12], l_012[n012 - v012:]])
                    g1 = np.concatenate(
                        [l_01[u01:], l_012[u012:n012 - v012], l_12[v12:]])
                    for w, lst in enumerate((g0, g1, g2)):
                        if len(lst) == 0:
                            continue
                        Kw = Ks[w]
                        j0 = ioffs[w] * 16 + (trel * Kw) * 128 + p
                        flat[c, j0:j0 + len(lst) * 128:128] = lst - WSTARTS[w]
                        mc = moff_g + boffs[w] + trel * Kw
                        mask[c, p, mc:mc + len(lst)] = 1.0

    # wrap each gather chunk's flat j-order into its 16-partition layout,
    # replicated 8x across the 128 partitions
    idxw = np.zeros((NC, 128, IW), np.int16)
    for c in range(NC):
        blk = flat[c]
        for (t0, G, Ks, ioffs, boffs, moff_g, chunks) in meta:
            for (w, bo, nc_, io) in chunks:
                nidx = 128 * nc_
                f = blk[io * 16: io * 16 + nidx]
                wr = f.reshape(-1, 16).T        # [16, nidx/16]
                idxw[c, :, io:io + nidx // 16] = np.tile(wr, (8, 1))

    return core, loc, meta, C3p, IW, idxw, mask


def _build(meta, C3p, IW):
    import concourse.bass as bass
    import concourse.bacc as bacc
    import concourse.mybir as mybir
    from concourse.tile import TileContext
    from concourse.masks import make_identity

    _patch_tile_drain()

    nc = bacc.Bacc("TRN2", target_bir_lowering=False, debug=False, num_devices=NC)
    f32 = mybir.dt.float32
    bf = mybir.dt.bfloat16
    AF = mybir.ActivationFunctionType

    xin = nc.dram_tensor("xin", [25, NPCP], f32, kind="ExternalInput")
    idx = nc.dram_tensor("idx", [128, IW], mybir.dt.int16, kind="ExternalInput")
    msk = nc.dram_tensor("msk", [128, C3p], bf, kind="ExternalInput")
    pTs = [nc.dram_tensor(f"pT{l}", [LAYERS[l][0], LAYERS[l][1] * LAYERS[l][2] + 2 * LAYERS[l][1]],
                          f32, kind="ExternalInput") for l in range(3)]
    bts = [nc.dram_tensor(f"bias{l}", [128, LAYERS[l][1] * LAYERS[l][2]],
                          f32, kind="ExternalInput") for l in range(3)]
    fcT = nc.dram_tensor("fcT", [50, 93], f32, kind="ExternalInput")
    fcb = nc.dram_tensor("fcb", [93, 1], f32, kind="ExternalInput")
    out93 = nc.dram_tensor("out93", [93, NPCP], f32, kind="ExternalOutput")

    pieces = [nc.dram_tensor(f"piece{l}", [NPCP, ELEM], bf, kind="Internal")
              for l in range(3)]
    tables = [nc.dram_tensor(f"table{l}", [NTOT, ELEM], bf, kind="Internal",
                             addr_space="Shared") for l in range(3)]

    with TileContext(nc) as tc:
        with (
            tc.tile_pool(name="const", bufs=1) as cpool,
            tc.tile_pool(name="gt", bufs=2) as gtp,
            tc.tile_pool(name="wk", bufs=2) as wkp,
            tc.tile_pool(name="ps", bufs=2, space="PSUM") as psp,
        ):
            ident = cpool.tile([128, 128], f32, tag="ident")
            make_identity(nc, ident[:])
            it = cpool.tile([128, IW], mybir.dt.int16, tag="idx")
            nc.sync.dma_start(it[:], idx[:])
            mk = cpool.tile([128, C3p], bf, tag="msk")
            nc.sync.dma_start(mk[:], msk[:])
            xin_sb = cpool.tile([25, NPCP], f32, tag="xin")
            nc.sync.dma_start(xin_sb[:], xin[:])
            pt_t = []
            b_t = []
            for l, (Fin, H, D) in enumerate(LAYERS):
                HD = H * D
                p = cpool.tile([Fin, HD + 2 * H], f32, tag=f"pt{l}")
                nc.sync.dma_start(p[:], pTs[l][:])
                pt_t.append(p)
                b = cpool.tile([128, HD], f32, tag=f"b{l}")
                nc.sync.dma_start(b[:], bts[l][:])
                b_t.append(b)
            fct = cpool.tile([50, 93], f32, tag="fct")
            nc.sync.dma_start(fct[:], fcT[:])
            fcbt = cpool.tile([93, 1], f32, tag="fcbt")
            nc.sync.dma_start(fcbt[:], fcb[:])
            ers = [cpool.tile([128, NT, LAYERS[l][1]], bf, tag=f"er{l}",
                              name=f"er{l}") for l in range(3)]

            def project(l, rhs_ap, t):
                """matmul -> transposed [node, feat] bf16 piece row write."""
                Fin, H, D = LAYERS[l]
                HD = H * D
                DR = HD + H
                PR = HD + 2 * H
                cp = psp.tile([PR, 128], f32, tag="ps_a", space="PSUM")
                nc.tensor.matmul(cp[:], lhsT=pt_t[l][:], rhs=rhs_ap,
                                 start=True, stop=True)
                cs = wkp.tile([PR, 128], f32, tag="cs")
                nc.scalar.activation(cs[:], cp[:], AF.Copy)
                gp = psp.tile([128, PR], f32, tag="ps_b", space="PSUM")
                nc.tensor.transpose(gp[:], cs[:], ident[:PR, :PR])
                gsb = wkp.tile([128, PR], bf, tag="gsb")
                nc.scalar.activation(gsb[:], gp[:], AF.Copy)
                nc.sync.dma_start(pieces[l][t * 128:(t + 1) * 128, 0:DR],
                                  gsb[:, 0:DR])
                nc.vector.tensor_copy(ers[l][:, t, :], gsb[:, DR:PR])

            def halo(l):
                nc.gpsimd.collective_compute(
                    "AllGather", mybir.AluOpType.bypass,
                    replica_groups=[list(range(NC))],
                    ins=[pieces[l][:]], outs=[tables[l][:]],
                )

            # ---- layer-0 projection + halo ----
            for t in range(NT):
                project(0, xin_sb[:, t * 128:(t + 1) * 128], t)
            halo(0)

            # ---- per-layer fused edge + next-projection ----
            for l, (Fin, H, D) in enumerate(LAYERS):
                HD = H * D
                table = tables[l]
                er_sb = ers[l]
                for (t0, G, Ks, ioffs, boffs, moff, chunks) in meta:
                    C3g = G * sum(Ks)
                    gbuf = gtp.tile([128, C3g, ELEM], bf, tag="gt")
                    for (w, bo, nc_, io) in chunks:
                        nidx = 128 * nc_
                        nc.gpsimd.dma_gather(
                            out_ap=gbuf[:, bo:bo + nc_, :],
                            in_ap=table[WSTARTS[w]:WSTARTS[w] + WIN, :],
                            idxs_ap=it[:, io:io + nidx // 16],
                            num_idxs=nidx, num_idxs_reg=nidx,
                            elem_size=ELEM,
                            single_packet=SINGLE_PACKET,
                        )
                    dens = []
                    accs = []
                    for w in range(3):
                        Kw = Ks[w]
                        if Kw == 0:
                            continue
                        cw = G * Kw
                        bw = boffs[w]
                        ew = wkp.tile([128, cw, H], bf, tag="e")
                        nc.vector.tensor_tensor(
                            out=ew[:].rearrange("p (g k) h -> p g k h", g=G),
                            in0=gbuf[:, bw:bw + cw, HD:HD + H].rearrange(
                                "p (g k) h -> p g k h", g=G),
                            in1=er_sb[:, t0:t0 + G, :].unsqueeze(2)
                                .broadcast_to([128, G, Kw, H]),
                            op=mybir.AluOpType.add)
                        nc.scalar.activation(ew[:], ew[:], AF.Prelu,
                                             alpha=NEG_SLOPE)
                        nc.scalar.activation(ew[:], ew[:], AF.Exp)
                        nc.vector.tensor_tensor(
                            out=ew[:], in0=ew[:],
                            in1=mk[:, moff + bw:moff + bw + cw].unsqueeze(2)
                                .broadcast_to([128, cw, H]),
                            op=mybir.AluOpType.mult)
                        dw = wkp.tile([128, G, H], f32, tag=f"d{w}",
                                      name=f"dw{w}")
                        nc.vector.tensor_reduce(
                            out=dw[:],
                            in_=ew[:].rearrange("p (g k) h -> p g h k", g=G),
                            axis=mybir.AxisListType.X, op=mybir.AluOpType.add)
                        dens.append(dw)
                        mw = wkp.tile([128, cw, HD], bf, tag="m")
                        nc.vector.tensor_tensor(
                            out=mw[:].rearrange(
                                "p (g k) (h d) -> p g k h d", g=G, h=H),
                            in0=gbuf[:, bw:bw + cw, 0:HD].rearrange(
                                "p (g k) (h d) -> p g k h d", g=G, h=H),
                            in1=ew[:].rearrange(
                                "p (g k) h -> p g k h", g=G).unsqueeze(4)
                                .broadcast_to([128, G, Kw, H, D]),
                            op=mybir.AluOpType.mult)
                        aw = wkp.tile([128, G, HD], f32, tag=f"a{w}",
                                      name=f"aw{w}")
                        nc.vector.tensor_reduce(
                            out=aw[:],
                            in_=mw[:].rearrange("p (g k) f -> p g f k", g=G),
                            axis=mybir.AxisListType.X, op=mybir.AluOpType.add)
                        accs.append(aw)
                    den = dens[0]
                    for dw in dens[1:]:
                        nc.vector.tensor_add(den[:], den[:], dw[:])
                    acc = accs[0]
                    for aw in accs[1:]:
                        nc.vector.tensor_add(acc[:], acc[:], aw[:])
                    nc.vector.tensor_scalar_max(den[:], den[:], 1e-30)
                    rden = wkp.tile([128, G, H], f32, tag="rden")
                    nc.vector.reciprocal(rden[:], den[:])
                    o = wkp.tile([128, G, HD], f32, tag="o")
                    nc.vector.tensor_tensor(
                        out=o[:].rearrange("p g (h d) -> p g h d", h=H),
                        in0=acc[:].rearrange("p g (h d) -> p g h d", h=H),
                        in1=rden[:].unsqueeze(3).broadcast_to([128, G, H, D]),
                        op=mybir.AluOpType.mult)
                    nc.vector.tensor_tensor(
                        out=o[:], in0=o[:],
                        in1=b_t[l][:].unsqueeze(1).broadcast_to([128, G, HD]),
                        op=mybir.AluOpType.add)
                    nc.vector.tensor_scalar_max(o[:], o[:], 0.0)

                    for trel in range(G):
                        t = t0 + trel
                        xp = psp.tile([HD, 128], f32, tag="ps_t", space="PSUM")
                        nc.tensor.transpose(xp[:], o[:, trel, :], ident[:])
                        xs2 = wkp.tile([HD, 128], f32, tag="xs2")
                        nc.scalar.activation(xs2[:], xp[:], AF.Copy)
                        if l < 2:
                            project(l + 1, xs2[:], t)
                        else:
                            fp = psp.tile([93, 128], f32, tag="ps_a",
                                          space="PSUM")
                            nc.tensor.matmul(fp[:], lhsT=fct[:], rhs=xs2[:],
                                             start=True, stop=True)
                            fo = wkp.tile([93, 128], f32, tag="fo")
                            nc.vector.tensor_tensor(
                                out=fo[:], in0=fp[:],
                                in1=fcbt[:, 0:1].broadcast_to([93, 128]),
                                op=mybir.AluOpType.add)
                            nc.sync.dma_start(out93[:, t * 128:(t + 1) * 128],
                                              fo[:])
                if l < 2:
                    halo(l + 1)

    nc.compile()
    return nc


def _proj_matrix(W, al, ar):
    """P = [W; L^T W; R^T W] so that P @ x = [h; el; er] (feature-major)."""
    H, D = al.shape
    HD = H * D
    L = np.zeros((HD, H), np.float32)
    R = np.zeros((HD, H), np.float32)
    for h in range(H):
        L[h * D:(h + 1) * D, h] = al[h]
        R[h * D:(h + 1) * D, h] = ar[h]
    return np.vstack([W, L.T @ W, R.T @ W]).astype(np.float32)


def _prepare(inputs):
    src = np.ascontiguousarray(np.asarray(inputs["src"], dtype=np.int64))
    dst = np.ascontiguousarray(np.asarray(inputs["dst"], dtype=np.int64))
    feats = np.asarray(inputs["features"], dtype=np.float32)

    core, loc, meta, C3p, IW, idxw, mask = _preprocess(src, dst)

    ck = (tuple((t0, G, Ks) for (t0, G, Ks, _, _, _, _) in meta), C3p, IW)
    if ck not in _cache:
        _cache[ck] = _build(meta, C3p, IW)
    nc = _cache[ck]

    pTl = []
    btl = []
    for l in range(3):
        W = np.asarray(inputs[f"W{l + 1}"], np.float32)
        al = np.asarray(inputs[f"al{l + 1}"], np.float32)
        ar = np.asarray(inputs[f"ar{l + 1}"], np.float32)
        b = np.asarray(inputs[f"b{l + 1}"], np.float32)
        P = _proj_matrix(W, al, ar)
        pTl.append(np.ascontiguousarray(P.T))
        btl.append(np.ascontiguousarray(np.tile(b[None, :], (128, 1))))
    fcw = np.asarray(inputs["fc_w"], np.float32)
    fcb = np.asarray(inputs["fc_b"], np.float32).reshape(93, 1)
    fcT = np.ascontiguousarray(fcw.T)

    in_maps = []
    for c in range(NC):
        xfm = np.zeros((25, NPCP), np.float32)
        sel = core == c
        xfm[:, loc[sel]] = feats[sel].T
        m = {"xin": xfm,
             "idx": np.ascontiguousarray(idxw[c]),
             "msk": np.ascontiguousarray(mask[c].astype(ml_dtypes.bfloat16)),
             "fcT": fcT, "fcb": fcb}
        for l in range(3):
            m[f"pT{l}"] = pTl[l]
            m[f"bias{l}"] = btl[l]
        in_maps.append(m)
    return nc, in_maps, (core, loc)


def kernel(**inputs):
    from concourse import bass_utils

    nc, in_maps, (core, loc) = _prepare(inputs)
    res = bass_utils.run_bass_kernel_spmd(nc, in_maps, core_ids=list(range(NC)))

    out = np.zeros((N_NODES, 93), np.float32)
    for c in range(NC):
        o = res.results[c]["out93"]          # [93, NPCP]
        sel = core == c
        out[np.where(sel)[0]] = o[:, loc[sel]].T
    return out
